# revision 1
# baseline (speedup 1.0000x reference)
"""Trainium2 Bass kernel for GRU regressor (B=256, T=512, F=64, H=512).

Data-parallel: batch sharded 32/core across 8 NeuronCores. Gate-major
transposed layout: state h kept as [128 partitions, 4 k-chunks x 32 batch]
(hidden unit u = k*128+p). Per step, each gate-row chunk accumulates in PSUM:
4 bf16 [128,128] W_hh chunks (moving operand = state, N=32) plus an augmented
K=65 W_ih chunk (64 features + ones-row carrying the biases) against the
per-step x column block, so sigmoid/tanh read complete pre-activations
straight from PSUM. Elementwise runs on [128, small] tiles on DVE/ACT.
The head matmul runs on host in fp32.
"""
import numpy as np

B, T, F, H = 256, 512, 64, 512
NCORES = 8
BC = B // NCORES          # 32 batch per core
NM = 12                   # 3H/128 gate-row chunks (0-3 r, 4-7 z, 8-11 n)
NK = 4                    # H/128 state chunks
FA = F + 1                # augmented contraction (features + bias row)

_cache = {}


def _build(Tsteps):
    import concourse.bass as bass
    import concourse.mybir as mybir
    from concourse.tile import TileContext
    from concourse.vector_clock import ScopedClock
    from bass_rust import SyncInfo

    MAXW = 1  # walrus TPB sync-wait slots per instruction

    class TC(TileContext):
        # walrus rejects >MAXW sync waits on one instruction; hoist the excess
        # onto same-engine NOPs inserted right before the offender.
        def _split_waits(self):
            nc = self.nc
            cur = nc.cur_bb.bb
            for fn in nc.m.functions:
                for bb in fn.blocks:
                    insts = bb.instructions
                    if not any(
                        i.sync_info and len(i.sync_info.on_wait) > MAXW
                        for i in insts
                    ):
                        continue
                    new_l = []
                    for inst in insts:
                        si = inst.sync_info
                        w = list(si.on_wait) if si else []
                        if len(w) > MAXW:
                            keep, excess = w[:MAXW], w[MAXW:]
                            for j in range(0, len(excess), MAXW):
                                nop = nc.engines[inst.engine].nop().ins
                                assert cur.instructions.pop() is nop
                                nop.sync_info = SyncInfo(
                                    on_wait=excess[j:j + MAXW], on_update=[])
                                new_l.append(nop)
                            inst.sync_info = SyncInfo(
                                on_wait=keep, on_update=list(si.on_update))
                        new_l.append(inst)
                    bb.instructions[:] = new_l

        def _drain_and_barrier(self, tick_clock, wait_clock):
            drain_inst = self.nc.sync.drain()
            wait_clock.add_sem_waits(
                drain_inst.ins, ScopedClock({None: tick_clock.global_clock})
            )
            self._split_waits()
            self.nc.all_engine_barrier()
            popped = self.nc._tile_sem_poison_stack.pop()
            assert popped is self._sem_poison
            self.nc.clear_and_free_semaphores(list(self.sems.allocated().values()))
            self.nc.all_engine_barrier()

    dt = mybir.dt
    AF = mybir.ActivationFunctionType
    nc = bass.Bass("TRN2", target_bir_lowering=False, debug=False,
                   num_devices=NCORES)

    xT = nc.declare_dram_parameter("xT", [FA, Tsteps * BC], dt.bfloat16, isOutput=False)
    Whh = nc.declare_dram_parameter("Whh", [128, NM * NK * 128], dt.bfloat16, isOutput=False)
    Wih = nc.declare_dram_parameter("Wih", [FA, NM * 128], dt.bfloat16, isOutput=False)
    Bnr = nc.declare_dram_parameter("Bnr", [1, NK * 128], dt.bfloat16, isOutput=False)
    hout = nc.declare_dram_parameter("hout", [128, NK * BC], dt.bfloat16, isOutput=True)

    with TC(nc) as tc:
        with (
            tc.tile_pool(name="const", bufs=1) as constp,
            tc.tile_pool(name="pr", bufs=2, space="PSUM") as prp,
            tc.tile_pool(name="pz", bufs=2, space="PSUM") as pzp,
            tc.tile_pool(name="pn", bufs=2, space="PSUM") as pnp,
            tc.tile_pool(name="pgn", bufs=2, space="PSUM") as pgnp,
            tc.tile_pool(name="ew", bufs=3) as ewp,
        ):
            whh_sb = constp.tile([128, NM * NK * 128], dt.bfloat16, tag="whh")
            wih_sb = constp.tile([FA, NM * 128], dt.bfloat16, tag="wih")
            xt_sb = constp.tile([FA, Tsteps * BC], dt.bfloat16, tag="xt")
            bnr_sb = constp.tile([1, NK * 128], dt.bfloat16, tag="bnr")
            ones_sb = constp.tile([1, BC], dt.bfloat16, tag="ones")
            ones_h = constp.tile([128, NK * BC], dt.bfloat16, tag="onesh")
            h_bf = constp.tile([128, NK * BC], dt.bfloat16, tag="h")

            nc.sync.dma_start(out=whh_sb[:], in_=Whh[:])
            nc.sync.dma_start(out=wih_sb[:], in_=Wih[:])
            nc.sync.dma_start(out=xt_sb[:], in_=xT[:])
            nc.sync.dma_start(out=bnr_sb[:], in_=Bnr[:])
            nc.gpsimd.memset(ones_sb[:], 1.0)
            nc.gpsimd.memset(ones_h[:], 1.0)
            nc.gpsimd.memset(h_bf[:], 0.0)

            def gate_group(o, m, xs, last):
                for k in range(NK):
                    nc.tensor.matmul(
                        o, whh_sb[:, (m * NK + k) * 128:(m * NK + k + 1) * 128],
                        h_bf[:, k * BC:(k + 1) * BC],
                        start=(k == 0), stop=False)
                nc.tensor.matmul(o, *last, start=False, stop=True)

            for t in range(Tsteps):
                xs = xt_sb[:, t * BC:(t + 1) * BC]
                pr = prp.tile([128, NK * BC], dt.float32, tag="pr")
                pz = pzp.tile([128, NK * BC], dt.float32, tag="pz")
                pn = pnp.tile([128, NK * BC], dt.float32, tag="pn")
                pgn = pgnp.tile([128, NK * BC], dt.float32, tag="pgn")
                # r-gate first: the critical chain starts at sigmoid(r)
                for m in range(4):
                    gate_group(pr[:, m * BC:(m + 1) * BC], m,
                               xs, (wih_sb[:, m * 128:(m + 1) * 128], xs))
                # n-gate next (needed by t2 right after sigmoid-r)
                for m in range(8, NM):
                    gate_group(pn[:, (m - 8) * BC:(m - 7) * BC], m, xs,
                               (bnr_sb[:, (m - 8) * 128:(m - 7) * 128], ones_sb[:]))
                    nc.tensor.matmul(
                        pgn[:, (m - 8) * BC:(m - 7) * BC],
                        wih_sb[:, m * 128:(m + 1) * 128], xs,
                        start=True, stop=True)
                # z-gate last: only needed once tanh is in flight
                for m in range(4, 8):
                    gate_group(pz[:, (m - 4) * BC:(m - 3) * BC], m,
                               xs, (wih_sb[:, m * 128:(m + 1) * 128], xs))
                HW = NK * BC
                sigr = ewp.tile([128, HW], dt.bfloat16, tag="sigr")
                nc.scalar.activation(sigr[:], pr[:], AF.Sigmoid)
                t2 = ewp.tile([128, HW], dt.bfloat16, tag="t2")
                nc.vector.tensor_mul(t2[:], sigr[:], pn[:])
                t3 = ewp.tile([128, HW], dt.bfloat16, tag="t3")
                nc.vector.tensor_add(t3[:], t2[:], pgn[:])
                # z-path off the critical chain: z, u=z*h, oz=1-z during tanh
                sigz = ewp.tile([128, HW], dt.bfloat16, tag="sigz")
                nc.scalar.activation(sigz[:], pz[:], AF.Sigmoid)
                u = ewp.tile([128, HW], dt.bfloat16, tag="u")
                nc.vector.tensor_mul(u[:], sigz[:], h_bf[:])
                oz = ewp.tile([128, HW], dt.bfloat16, tag="oz")
                nc.vector.tensor_sub(oz[:], ones_h[:], sigz[:])
                nt = ewp.tile([128, HW], dt.bfloat16, tag="nt")
                nc.scalar.activation(nt[:], t3[:], AF.Tanh)
                v = ewp.tile([128, HW], dt.bfloat16, tag="v")
                nc.vector.tensor_mul(v[:], oz[:], nt[:])
                nc.vector.tensor_add(h_bf[:], u[:], v[:])

            nc.sync.dma_start(out=hout[:], in_=h_bf[:])
    return nc


def kernel(x, W_ih, W_hh, b_ih, b_hh, head_w, head_b):
    import ml_dtypes
    from concourse.bass_utils import run_bass_kernel_spmd

    Tsteps = x.shape[1]
    if Tsteps not in _cache:
        _cache[Tsteps] = _build(Tsteps)
    nc = _cache[Tsteps]

    bf16 = ml_dtypes.bfloat16
    whh = np.ascontiguousarray(
        np.transpose(W_hh.reshape(NM, 128, NK, 128), (3, 0, 2, 1))
    ).reshape(128, NM * NK * 128).astype(bf16)
    # augmented W_ih: feature rows + bias row (b_ih+b_hh for r/z, b_ih for n)
    wih = np.empty((FA, NM * 128), np.float32)
    wih[:F] = W_ih.T
    ball = b_ih + b_hh
    wih[F, :8 * 128] = ball[:8 * 128]
    wih[F, 8 * 128:] = b_ih[8 * 128:]
    wih = wih.astype(bf16)
    bnr = b_hh[2 * H:3 * H].reshape(1, NK * 128).astype(bf16)

    in_maps = []
    for ci in range(NCORES):
        xs = x[ci * BC:(ci + 1) * BC]               # [BC, T, F]
        xt = np.empty((FA, Tsteps, BC), np.float32)
        xt[:F] = np.transpose(xs, (2, 1, 0))
        xt[F] = 1.0
        xt = xt.reshape(FA, Tsteps * BC).astype(bf16)
        in_maps.append({"xT": xt, "Whh": whh, "Wih": wih, "Bnr": bnr})

    res = run_bass_kernel_spmd(nc, in_maps, list(range(NCORES)))
    kernel.last_results = res
    kernel.last_in_maps = in_maps

    h_full = np.empty((B, H), np.float32)
    for ci in range(NCORES):
        hl = np.asarray(res.results[ci]["hout"], np.float32)  # [p, k*BC]
        hl = hl.reshape(128, NK, BC)
        h_full[ci * BC:(ci + 1) * BC] = np.transpose(hl, (2, 1, 0)).reshape(BC, H)

    y = h_full @ head_w.T.astype(np.float32) + head_b
    return y.squeeze(-1).astype(np.float32)



# revision 2
# speedup vs baseline: 1.0692x; 1.0692x over previous
"""Trainium2 Bass kernel for GRU regressor (B=256, T=512, F=64, H=512).

Data-parallel: batch sharded 32/core across 8 NeuronCores. Gate-major
transposed layout: state h kept as [128 partitions, 4 k-chunks x 32 batch]
(hidden unit u = k*128+p). Per step, each gate-row chunk accumulates in PSUM:
4 bf16 [128,128] W_hh chunks (moving operand = state, N=32) plus an augmented
K=65 W_ih chunk (64 features + ones-row carrying the biases) against the
per-step x column block, so sigmoid/tanh read complete pre-activations
straight from PSUM. Elementwise runs on [128, small] tiles on DVE/ACT.

Host-side cost dominates this problem (axon tunnel: ~60-80ms fixed per
transferred array + ~8.6ms/MB, plus per-call jit re-trace), so:
- the timestep loop is a hardware For_i loop (register-offset x slicing),
  keeping the module ~200 instructions instead of ~37k unrolled;
- x ships as int8 (per-feature scales folded into W_ih on host; device
  converts int8->bf16 exactly);
- weights ship as per-core 1/8 shards, AllGathered on device;
- everything rides in ONE packed bf16 blob per core (int8 x region viewed
  via bitcast) to pay the fixed transfer latency once.
The head matmul runs on host in fp32.
"""
import numpy as np

B, T, F, H = 256, 512, 64, 512
NCORES = 8
BC = B // NCORES          # 32 batch per core
NM = 12                   # 3H/128 gate-row chunks (0-3 r, 4-7 z, 8-11 n)
NK = 4                    # H/128 state chunks
FA = F + 1                # augmented contraction (features + bias row)

_cache = {}


def _build(Tsteps):
    import concourse.bass as bass
    import concourse.mybir as mybir
    from concourse.bass import ts
    from concourse.tile import TileContext
    from concourse.vector_clock import ScopedClock
    from bass_rust import SyncInfo

    MAXW = 1  # walrus TPB sync-wait slots per instruction

    class TC(TileContext):
        # walrus rejects >MAXW sync waits on one instruction; hoist the excess
        # onto same-engine NOPs inserted right before the offender.
        def _split_waits(self):
            nc = self.nc
            cur = nc.cur_bb.bb
            for fn in nc.m.functions:
                for bb in fn.blocks:
                    insts = bb.instructions
                    if not any(
                        i.sync_info and len(i.sync_info.on_wait) > MAXW
                        for i in insts
                    ):
                        continue
                    new_l = []
                    for inst in insts:
                        si = inst.sync_info
                        w = list(si.on_wait) if si else []
                        if len(w) > MAXW:
                            keep, excess = w[:MAXW], w[MAXW:]
                            for j in range(0, len(excess), MAXW):
                                nop = nc.engines[inst.engine].nop().ins
                                assert cur.instructions.pop() is nop
                                nop.sync_info = SyncInfo(
                                    on_wait=excess[j:j + MAXW], on_update=[])
                                new_l.append(nop)
                            inst.sync_info = SyncInfo(
                                on_wait=keep, on_update=list(si.on_update))
                        new_l.append(inst)
                    bb.instructions[:] = new_l

        def _drain_and_barrier(self, tick_clock, wait_clock):
            drain_inst = self.nc.sync.drain()
            wait_clock.add_sem_waits(
                drain_inst.ins, ScopedClock({None: tick_clock.global_clock})
            )
            self._split_waits()
            self.nc.all_engine_barrier()
            popped = self.nc._tile_sem_poison_stack.pop()
            assert popped is self._sem_poison
            self.nc.clear_and_free_semaphores(list(self.sems.allocated().values()))
            self.nc.all_engine_barrier()

    dt = mybir.dt
    AF = mybir.ActivationFunctionType
    nc = bass.Bass("TRN2", target_bir_lowering=False, debug=False,
                   num_devices=NCORES)

    NW = NM * NK * 128          # 6144 whh cols
    NI = NM * 128               # 1536 wih cols
    XR = FA                     # 65 int8 rows of x data
    XCB = Tsteps * BC           # 16384 int8 cols = 8192 bf16
    WROWS = 15                  # weight shard rows of 8192 bf16
    # blob rows (bf16 [*, 8192]): 0-64 x int8 (bitcast), 65-79 wsh, 80 bnr
    BR = XR + WROWS + 1         # 81

    blob = nc.declare_dram_parameter("blob", [BR, XCB // 2], dt.bfloat16, isOutput=False)
    hout = nc.declare_dram_parameter("hout", [128, NK * BC], dt.bfloat16, isOutput=True)
    shin = nc.dram_tensor("shin", [WROWS, XCB // 2], dt.bfloat16)
    gath = nc.dram_tensor("gath", [128, NW + NI], dt.bfloat16)
    blob8 = blob.bitcast(dt.int8)      # [81, 16384] int8 view

    with TC(nc) as tc:
        with (
            tc.tile_pool(name="const", bufs=1) as constp,
            tc.tile_pool(name="ps", bufs=1, space="PSUM") as psp,
        ):
            wsh_sb = constp.tile([WROWS, XCB // 2], dt.bfloat16, tag="wsh")
            xq_sb = constp.tile([FA, Tsteps * BC], dt.int8, tag="xq")
            whh_sb = constp.tile([128, NW], dt.bfloat16, tag="whh")
            wih_sb = constp.tile([FA, NI], dt.bfloat16, tag="wih")
            xt_sb = constp.tile([FA, Tsteps * BC], dt.bfloat16, tag="xt")
            bnr_sb = constp.tile([1, NK * 128], dt.bfloat16, tag="bnr")
            ones_sb = constp.tile([1, BC], dt.bfloat16, tag="ones")
            ones_h = constp.tile([128, NK * BC], dt.bfloat16, tag="onesh")
            h_bf = constp.tile([128, NK * BC], dt.bfloat16, tag="h")
            HW = NK * BC
            sigr = constp.tile([128, HW], dt.bfloat16, tag="sigr")
            t2 = constp.tile([128, HW], dt.bfloat16, tag="t2")
            t3 = constp.tile([128, HW], dt.bfloat16, tag="t3")
            sigz = constp.tile([128, HW], dt.bfloat16, tag="sigz")
            u = constp.tile([128, HW], dt.bfloat16, tag="u")
            oz = constp.tile([128, HW], dt.bfloat16, tag="oz")
            nt = constp.tile([128, HW], dt.bfloat16, tag="nt")
            v = constp.tile([128, HW], dt.bfloat16, tag="v")
            pr = psp.tile([128, NK * BC], dt.float32, tag="pr")
            pz = psp.tile([128, NK * BC], dt.float32, tag="pz")
            pn = psp.tile([128, NK * BC], dt.float32, tag="pn")
            pgn = psp.tile([128, NK * BC], dt.float32, tag="pgn")

            # weights: blob shard rows -> SBUF -> internal DRAM -> AllGather
            nc.sync.dma_start(out=wsh_sb[:], in_=blob[XR:XR + WROWS, :])
            nc.sync.dma_start(out=shin[:], in_=wsh_sb[:])
            nc.gpsimd.collective_compute(
                "AllGather", mybir.AluOpType.bypass,
                replica_groups=[list(range(NCORES))],
                ins=[shin[:].opt()], outs=[gath[:].opt()])
            nc.sync.dma_start(out=whh_sb[:], in_=gath[:, 0:NW])
            nc.sync.dma_start(out=wih_sb[:], in_=gath[0:FA, NW:NW + NI])
            # x: int8 region of the blob -> SBUF -> exact bf16 integers
            # (per-feature dequant scale is folded into W_ih on the host)
            nc.sync.dma_start(out=xq_sb[:], in_=blob8[0:XR, :])
            nc.vector.tensor_copy(xt_sb[:], xq_sb[:])
            nc.sync.dma_start(out=bnr_sb[:], in_=blob[XR + WROWS:BR, 0:NK * 128])
            nc.gpsimd.memset(ones_sb[:], 1.0)
            nc.gpsimd.memset(ones_h[:], 1.0)
            nc.gpsimd.memset(h_bf[:], 0.0)

            def gate_group(o, m, xs, last):
                for k in range(NK):
                    nc.tensor.matmul(
                        o, whh_sb[:, (m * NK + k) * 128:(m * NK + k + 1) * 128],
                        h_bf[:, k * BC:(k + 1) * BC],
                        start=(k == 0), stop=False)
                nc.tensor.matmul(o, *last, start=False, stop=True)

            with tc.For_i(0, Tsteps, 1) as i:
                xs = xt_sb[:, ts(i, BC)]
                # r-gate first: the critical chain starts at sigmoid(r)
                for m in range(4):
                    gate_group(pr[:, m * BC:(m + 1) * BC], m,
                               xs, (wih_sb[:, m * 128:(m + 1) * 128], xs))
                # n-gate next (needed by t2 right after sigmoid-r)
                for m in range(8, NM):
                    gate_group(pn[:, (m - 8) * BC:(m - 7) * BC], m, xs,
                               (bnr_sb[:, (m - 8) * 128:(m - 7) * 128], ones_sb[:]))
                    nc.tensor.matmul(
                        pgn[:, (m - 8) * BC:(m - 7) * BC],
                        wih_sb[:, m * 128:(m + 1) * 128], xs,
                        start=True, stop=True)
                # z-gate last: only needed once tanh is in flight
                for m in range(4, 8):
                    gate_group(pz[:, (m - 4) * BC:(m - 3) * BC], m,
                               xs, (wih_sb[:, m * 128:(m + 1) * 128], xs))
                nc.scalar.activation(sigr[:], pr[:], AF.Sigmoid)
                nc.vector.tensor_mul(t2[:], sigr[:], pn[:])
                nc.vector.tensor_add(t3[:], t2[:], pgn[:])
                # z-path off the critical chain: z, u=z*h, oz=1-z during tanh
                nc.scalar.activation(sigz[:], pz[:], AF.Sigmoid)
                nc.vector.tensor_mul(u[:], sigz[:], h_bf[:])
                nc.vector.tensor_sub(oz[:], ones_h[:], sigz[:])
                nc.scalar.activation(nt[:], t3[:], AF.Tanh)
                nc.vector.tensor_mul(v[:], oz[:], nt[:])
                nc.vector.tensor_add(h_bf[:], u[:], v[:])

            nc.sync.dma_start(out=hout[:], in_=h_bf[:])
    return nc


def kernel(x, W_ih, W_hh, b_ih, b_hh, head_w, head_b):
    import ml_dtypes
    from concourse.bass_utils import run_bass_kernel_spmd

    Tsteps = x.shape[1]
    if Tsteps not in _cache:
        _cache[Tsteps] = _build(Tsteps)
    nc = _cache[Tsteps]

    bf16 = ml_dtypes.bfloat16
    NW = NM * NK * 128
    NI = NM * 128
    SR = 128 // NCORES          # 16 rows of the [128, 7680] weight pack
    whh = np.ascontiguousarray(
        np.transpose(W_hh.reshape(NM, 128, NK, 128), (3, 0, 2, 1))
    ).reshape(128, NW)
    # per-feature int8 quantization of x; dequant scale folded into W_ih
    xf = np.asarray(x, np.float32)
    scale = np.abs(xf).max(axis=(0, 1)) / 127.0          # [F]
    scale[scale == 0] = 1.0
    # augmented W_ih: feature rows + bias row (b_ih+b_hh for r/z, b_ih for n)
    wih = np.zeros((128, NI), np.float32)
    wih[:F] = W_ih.T * scale[:, None]
    ball = b_ih + b_hh
    wih[F, :8 * 128] = ball[:8 * 128]
    wih[F, 8 * 128:] = b_ih[8 * 128:]
    wpack = np.concatenate([whh, wih], axis=1).astype(bf16)  # [128, 7680]
    bnr = b_hh[2 * H:3 * H].astype(bf16)                     # [512]

    xq = np.rint(xf / scale[None, None, :]).astype(np.int8)  # [B, T, F]
    XBYTES = FA * Tsteps * BC                                # 1064960
    WBYTES = SR * (NW + NI) * 2                              # 245760
    ROWB = Tsteps * BC                                       # 16384 B/row
    NROWS = FA + WBYTES // ROWB + 1                          # 81
    in_maps = []
    for ci in range(NCORES):
        xs = xq[ci * BC:(ci + 1) * BC]              # [BC, T, F] int8
        xt = np.empty((FA, Tsteps, BC), np.int8)
        xt[:F] = np.transpose(xs, (2, 1, 0))
        xt[F] = 1
        raw = np.zeros(NROWS * ROWB, np.uint8)
        raw[:XBYTES] = xt.reshape(-1).view(np.uint8)
        raw[XBYTES:XBYTES + WBYTES] = (
            wpack[ci * SR:(ci + 1) * SR].reshape(-1).view(np.uint8))
        raw[XBYTES + WBYTES:XBYTES + WBYTES + 1024] = bnr.view(np.uint8)
        blob = raw.view(bf16).reshape(NROWS, ROWB // 2)
        in_maps.append({"blob": blob})

    res = run_bass_kernel_spmd(nc, in_maps, list(range(NCORES)))
    kernel.last_results = res
    kernel.last_in_maps = in_maps

    h_full = np.empty((B, H), np.float32)
    for ci in range(NCORES):
        hl = np.asarray(res.results[ci]["hout"], np.float32)  # [p, k*BC]
        hl = hl.reshape(128, NK, BC)
        h_full[ci * BC:(ci + 1) * BC] = np.transpose(hl, (2, 1, 0)).reshape(BC, H)

    y = h_full @ head_w.T.astype(np.float32) + head_b
    return y.squeeze(-1).astype(np.float32)


# revision 4
# speedup vs baseline: 1.0942x; 1.0234x over previous
"""Trainium2 Bass kernel for GRU regressor (B=256, T=512, F=64, H=512).

Data-parallel: batch sharded 32/core across 8 NeuronCores. Gate-major
transposed layout: state h kept as [128 partitions, 4 k-chunks x 32 batch]
(hidden unit u = k*128+p). Per step, each gate-row chunk accumulates in PSUM:
4 bf16 [128,128] W_hh chunks (moving operand = state, N=32) plus an augmented
K=65 W_ih chunk (64 features + ones-row carrying the biases) against the
per-step x column block, so sigmoid/tanh read complete pre-activations
straight from PSUM. Elementwise runs on [128, small] tiles on DVE/ACT.

Host-side cost dominates this problem (axon tunnel: ~60-80ms fixed per
transferred array + ~8.6ms/MB, plus per-call jit re-trace), so:
- the timestep loop is a hardware For_i loop (register-offset x slicing),
  keeping the module ~200 instructions instead of ~37k unrolled;
- x ships as int8 (per-feature scales folded into W_ih on host; device
  converts int8->bf16 exactly);
- weights ship as per-core 1/8 shards, AllGathered on device;
- everything rides in ONE packed bf16 blob per core (int8 x region viewed
  via bitcast) to pay the fixed transfer latency once.
The head matmul runs on host in fp32.
"""
import numpy as np

B, T, F, H = 256, 512, 64, 512
NCORES = 8
BC = B // NCORES          # 32 batch per core
NM = 12                   # 3H/128 gate-row chunks (0-3 r, 4-7 z, 8-11 n)
NK = 4                    # H/128 state chunks
FA = F + 1                # augmented contraction (features + bias row)

_cache = {}


def _build(Tsteps):
    import concourse.bass as bass
    import concourse.mybir as mybir
    from concourse.bass import ts
    from concourse.tile import TileContext
    from concourse.vector_clock import ScopedClock
    from bass_rust import SyncInfo

    MAXW = 1  # walrus TPB sync-wait slots per instruction

    class TC(TileContext):
        # walrus rejects >MAXW sync waits on one instruction; hoist the excess
        # onto same-engine NOPs inserted right before the offender.
        def _split_waits(self):
            nc = self.nc
            cur = nc.cur_bb.bb
            for fn in nc.m.functions:
                for bb in fn.blocks:
                    insts = bb.instructions
                    if not any(
                        i.sync_info and len(i.sync_info.on_wait) > MAXW
                        for i in insts
                    ):
                        continue
                    new_l = []
                    for inst in insts:
                        si = inst.sync_info
                        w = list(si.on_wait) if si else []
                        if len(w) > MAXW:
                            keep, excess = w[:MAXW], w[MAXW:]
                            for j in range(0, len(excess), MAXW):
                                nop = nc.engines[inst.engine].nop().ins
                                assert cur.instructions.pop() is nop
                                nop.sync_info = SyncInfo(
                                    on_wait=excess[j:j + MAXW], on_update=[])
                                new_l.append(nop)
                            inst.sync_info = SyncInfo(
                                on_wait=keep, on_update=list(si.on_update))
                        new_l.append(inst)
                    bb.instructions[:] = new_l

        def _drain_and_barrier(self, tick_clock, wait_clock):
            drain_inst = self.nc.sync.drain()
            wait_clock.add_sem_waits(
                drain_inst.ins, ScopedClock({None: tick_clock.global_clock})
            )
            self._split_waits()
            self.nc.all_engine_barrier()
            popped = self.nc._tile_sem_poison_stack.pop()
            assert popped is self._sem_poison
            self.nc.clear_and_free_semaphores(list(self.sems.allocated().values()))
            self.nc.all_engine_barrier()

    dt = mybir.dt
    AF = mybir.ActivationFunctionType
    nc = bass.Bass("TRN2", target_bir_lowering=False, debug=False,
                   num_devices=NCORES)

    NW = NM * NK * 128          # 6144 whh cols
    NI = NM * 128               # 1536 wih cols
    XR = FA                     # 65 int8 rows of x data
    XCB = Tsteps * BC           # 16384 int8 cols = 8192 bf16
    WROWS = 8                   # int8 weight-pack shard rows (8192 bf16 each)
    # blob rows (bf16 [*, 8192]): 0-64 x int8 (bitcast), 65-79 wsh, 80 bnr
    BR = XR + WROWS + 1         # 81

    # uint16, not bf16: the axon tunnel canonicalizes bf16 NaN bit patterns,
    # which corrupts packed int8 bytes that alias NaN encodings
    blob = nc.declare_dram_parameter("blob", [BR, XCB // 2], dt.uint16, isOutput=False)
    blobbf = blob.bitcast(dt.bfloat16)
    yout = nc.declare_dram_parameter("yout", [1, BC], dt.float32, isOutput=True)
    # uint16 end-to-end for the packed weight path: the AllGather (declared
    # dtype bf16) canonicalizes NaN-aliasing byte pairs, corrupting int8 data
    shin = nc.dram_tensor("shin", [WROWS, XCB // 2], dt.uint16)
    # gathered weight pack, one 8192-byte row per hidden unit u:
    # [whh_q int8 6144 | wih_q int8 1536 | s_w bf16 | s_i[u] bf16 | pad]
    gath = nc.dram_tensor("gath", [128, 4096], dt.uint16)
    g8 = gath.bitcast(dt.int8)
    gathbf = gath.bitcast(dt.bfloat16)
    blob8 = blob.bitcast(dt.int8)      # [81, 16384] int8 view

    with TC(nc) as tc:
        with (
            tc.tile_pool(name="const", bufs=1) as constp,
            tc.tile_pool(name="ps", bufs=1, space="PSUM") as psp,
        ):
            wsh_sb = constp.tile([WROWS, XCB // 2], dt.uint16, tag="wsh")
            wq_sb = constp.tile([128, NW + NI], dt.int8, tag="wq")
            scw = constp.tile([128, 1], dt.bfloat16, tag="scw")
            sci = constp.tile([FA, 1], dt.bfloat16, tag="sci")
            xq_sb = constp.tile([FA, Tsteps * BC], dt.int8, tag="xq")
            whh_sb = constp.tile([128, NW], dt.bfloat16, tag="whh")
            wih_sb = constp.tile([FA, NI], dt.bfloat16, tag="wih")
            xt_sb = constp.tile([FA, Tsteps * BC], dt.bfloat16, tag="xt")
            bnr_sb = constp.tile([1, NK * 128], dt.bfloat16, tag="bnr")
            ones_sb = constp.tile([1, BC], dt.bfloat16, tag="ones")
            ones_h = constp.tile([128, NK * BC], dt.bfloat16, tag="onesh")
            h_bf = constp.tile([128, NK * BC], dt.bfloat16, tag="h")
            HW = NK * BC
            sigr = constp.tile([128, HW], dt.bfloat16, tag="sigr")
            t2 = constp.tile([128, HW], dt.bfloat16, tag="t2")
            t3 = constp.tile([128, HW], dt.bfloat16, tag="t3")
            sigz = constp.tile([128, HW], dt.bfloat16, tag="sigz")
            u = constp.tile([128, HW], dt.bfloat16, tag="u")
            oz = constp.tile([128, HW], dt.bfloat16, tag="oz")
            nt = constp.tile([128, HW], dt.bfloat16, tag="nt")
            v = constp.tile([128, HW], dt.bfloat16, tag="v")
            pr = psp.tile([128, NK * BC], dt.float32, tag="pr")
            pz = psp.tile([128, NK * BC], dt.float32, tag="pz")
            pn = psp.tile([128, NK * BC], dt.float32, tag="pn")
            pgn = psp.tile([128, NK * BC], dt.float32, tag="pgn")
            py = psp.tile([1, BC], dt.float32, tag="py")
            hw_sb = constp.tile([128, NK], dt.bfloat16, tag="hw")
            hb_sb = constp.tile([1, 1], dt.bfloat16, tag="hb")
            y_sb = constp.tile([1, BC], dt.float32, tag="y")

            # weights: blob shard rows -> SBUF -> internal DRAM -> AllGather
            nc.sync.dma_start(out=wsh_sb[:], in_=blob[XR:XR + WROWS, :])
            nc.sync.dma_start(out=shin[:], in_=wsh_sb[:])
            nc.gpsimd.collective_compute(
                "AllGather", mybir.AluOpType.bypass,
                replica_groups=[list(range(NCORES))],
                ins=[shin[:].opt()], outs=[gath[:].opt()])
            nc.sync.dma_start(out=wq_sb[:], in_=g8[:, 0:NW + NI])
            nc.sync.dma_start(out=scw[:], in_=gathbf[:, 3840:3841])
            nc.sync.dma_start(out=sci[:], in_=gathbf[0:FA, 3841:3842])
            # int8 -> exact bf16 integers, then scale by the per-row column
            wraw = constp.tile([128, NW], dt.bfloat16, tag="wraw")
            # int8->bf16 tensor_copy mis-decodes negatives (sign bit read as
            # +128 bias) when the copy spans all 128 partitions; <=64-partition
            # copies decode correctly, so split in half
            nc.vector.tensor_copy(wraw[0:64, :], wq_sb[0:64, 0:NW])
            nc.vector.tensor_copy(wraw[64:128, :], wq_sb[64:128, 0:NW])
            # chunk the broadcast multiply: wide (6144) broadcast APs corrupt
            # silently; 1536-wide (= the working wih width) is safe
            for c in range(4):
                nc.vector.tensor_mul(
                    whh_sb[:, c * 1536:(c + 1) * 1536],
                    wraw[:, c * 1536:(c + 1) * 1536],
                    scw[:, 0:1].broadcast_to([128, 1536]))
            iraw = constp.tile([FA, NI], dt.bfloat16, tag="iraw")
            nc.vector.tensor_copy(iraw[:], wq_sb[0:FA, NW:NW + NI])
            nc.vector.tensor_mul(wih_sb[:], iraw[:],
                                 sci[:, 0:1].broadcast_to([FA, NI]))
            # x: int8 region of the blob -> SBUF -> exact bf16 integers
            # (per-feature dequant scale is folded into W_ih on the host)
            nc.sync.dma_start(out=xq_sb[:], in_=blob8[0:XR, :])
            nc.vector.tensor_copy(xt_sb[:], xq_sb[:])
            nc.sync.dma_start(out=bnr_sb[:], in_=blobbf[XR + WROWS:BR, 0:NK * 128])
            nc.sync.dma_start(out=hw_sb[:], in_=blobbf[XR + WROWS:BR, NK * 128:2 * NK * 128])
            nc.sync.dma_start(out=hb_sb[:], in_=blobbf[XR + WROWS:BR, 2 * NK * 128:2 * NK * 128 + 1])
            nc.gpsimd.memset(ones_sb[:], 1.0)
            nc.gpsimd.memset(ones_h[:], 1.0)
            nc.gpsimd.memset(h_bf[:], 0.0)

            def gate_group(o, m, xs, last):
                for k in range(NK):
                    nc.tensor.matmul(
                        o, whh_sb[:, (m * NK + k) * 128:(m * NK + k + 1) * 128],
                        h_bf[:, k * BC:(k + 1) * BC],
                        start=(k == 0), stop=False)
                nc.tensor.matmul(o, *last, start=False, stop=True)

            with tc.For_i(0, Tsteps, 1) as i:
                xs = xt_sb[:, ts(i, BC)]
                # r-gate first: the critical chain starts at sigmoid(r)
                for m in range(4):
                    gate_group(pr[:, m * BC:(m + 1) * BC], m,
                               xs, (wih_sb[:, m * 128:(m + 1) * 128], xs))
                # n-gate next (needed by t2 right after sigmoid-r)
                for m in range(8, NM):
                    gate_group(pn[:, (m - 8) * BC:(m - 7) * BC], m, xs,
                               (bnr_sb[:, (m - 8) * 128:(m - 7) * 128], ones_sb[:]))
                    nc.tensor.matmul(
                        pgn[:, (m - 8) * BC:(m - 7) * BC],
                        wih_sb[:, m * 128:(m + 1) * 128], xs,
                        start=True, stop=True)
                # z-gate last: only needed once tanh is in flight
                for m in range(4, 8):
                    gate_group(pz[:, (m - 4) * BC:(m - 3) * BC], m,
                               xs, (wih_sb[:, m * 128:(m + 1) * 128], xs))
                nc.scalar.activation(sigr[:], pr[:], AF.Sigmoid)
                nc.vector.tensor_mul(t2[:], sigr[:], pn[:])
                nc.vector.tensor_add(t3[:], t2[:], pgn[:])
                # z-path off the critical chain: z, u=z*h, oz=1-z during tanh
                nc.scalar.activation(sigz[:], pz[:], AF.Sigmoid)
                nc.vector.tensor_mul(u[:], sigz[:], h_bf[:])
                nc.vector.tensor_sub(oz[:], ones_h[:], sigz[:])
                nc.scalar.activation(nt[:], t3[:], AF.Tanh)
                nc.vector.tensor_mul(v[:], oz[:], nt[:])
                nc.vector.tensor_add(h_bf[:], u[:], v[:])

            for k in range(NK):
                nc.tensor.matmul(py[:], hw_sb[:, k:k + 1],
                                 h_bf[:, k * BC:(k + 1) * BC],
                                 start=(k == 0), stop=False)
            nc.tensor.matmul(py[:], hb_sb[:], ones_sb[:],
                             start=False, stop=True)
            nc.vector.tensor_copy(y_sb[:], py[:])
            nc.sync.dma_start(out=yout[:], in_=y_sb[:])
    return nc


def kernel(x, W_ih, W_hh, b_ih, b_hh, head_w, head_b):
    import ml_dtypes
    from concourse.bass_utils import run_bass_kernel_spmd

    Tsteps = x.shape[1]
    if Tsteps not in _cache:
        _cache[Tsteps] = _build(Tsteps)
    nc = _cache[Tsteps]

    bf16 = ml_dtypes.bfloat16
    NW = NM * NK * 128
    NI = NM * 128
    SR = 128 // NCORES          # 16 rows of the [128, 7680] weight pack
    whh = np.ascontiguousarray(
        np.transpose(W_hh.reshape(NM, 128, NK, 128), (3, 0, 2, 1))
    ).reshape(128, NW)
    # per-feature int8 quantization of x; dequant scale folded into W_ih
    xf = np.asarray(x, np.float32)
    scale = np.abs(xf).max(axis=(0, 1)) / 127.0          # [F]
    scale[scale == 0] = 1.0
    # augmented W_ih: feature rows + bias row (b_ih+b_hh for r/z, b_ih for n)
    wih = np.zeros((128, NI), np.float32)
    wih[:F] = W_ih.T * scale[:, None]
    ball = b_ih + b_hh
    wih[F, :8 * 128] = ball[:8 * 128]
    wih[F, 8 * 128:] = b_ih[8 * 128:]
    # int8 weight pack: per-row scales (whh shares one scale; wih rows vary
    # because the x-dequant scales are folded in and row 64 holds biases)
    s_w = np.float32(np.abs(whh).max() / 127.0)
    s_w = np.float32(bf16(s_w))                              # exact bf16 scale
    whh_q = np.clip(np.rint(whh / s_w), -127, 127).astype(np.int8)
    s_i = np.abs(wih).max(axis=1) / 127.0                    # [128]
    s_i[s_i == 0] = 1.0
    s_i = np.float32(s_i.astype(bf16))
    wih_q = np.clip(np.rint(wih / s_i[:, None]), -127, 127).astype(np.int8)
    wpack = np.zeros((128, 8192), np.uint8)                  # [128, 8KB rows]
    wpack[:, :NW] = whh_q.view(np.uint8)
    wpack[:, NW:NW + NI] = wih_q.view(np.uint8)
    wpack[:, 7680:7682] = np.frombuffer(
        np.full(128, s_w, dtype=bf16).tobytes(), np.uint8).reshape(128, 2)
    wpack[:, 7682:7684] = np.frombuffer(
        s_i.astype(bf16).tobytes(), np.uint8).reshape(128, 2)
    bnr = b_hh[2 * H:3 * H].astype(bf16)                     # [512]
    # head_w reordered so SBUF [128, NK] has hw[p, k] = head_w[k*128+p]
    hwr = np.ascontiguousarray(
        head_w.reshape(NK, 128).T).reshape(-1).astype(bf16)  # [512]
    hbv = np.asarray(head_b, np.float32).reshape(1).astype(bf16)

    xq = np.rint(xf / scale[None, None, :]).astype(np.int8)  # [B, T, F]
    XBYTES = FA * Tsteps * BC                                # 1064960
    WBYTES = SR * 8192                                       # 131072
    ROWB = Tsteps * BC                                       # 16384 B/row
    NROWS = FA + WBYTES // ROWB + 1                          # 81
    in_maps = []
    for ci in range(NCORES):
        xs = xq[ci * BC:(ci + 1) * BC]              # [BC, T, F] int8
        xt = np.empty((FA, Tsteps, BC), np.int8)
        xt[:F] = np.transpose(xs, (2, 1, 0))
        xt[F] = 1
        raw = np.zeros(NROWS * ROWB, np.uint8)
        raw[:XBYTES] = xt.reshape(-1).view(np.uint8)
        raw[XBYTES:XBYTES + WBYTES] = wpack[ci * SR:(ci + 1) * SR].reshape(-1)
        raw[XBYTES + WBYTES:XBYTES + WBYTES + 1024] = bnr.view(np.uint8)
        raw[XBYTES + WBYTES + 1024:XBYTES + WBYTES + 2048] = hwr.view(np.uint8)
        raw[XBYTES + WBYTES + 2048:XBYTES + WBYTES + 2050] = hbv.view(np.uint8)
        blob = raw.view(np.uint16).reshape(NROWS, ROWB // 2)
        in_maps.append({"blob": blob})

    res = run_bass_kernel_spmd(nc, in_maps, list(range(NCORES)))
    kernel.last_results = res
    kernel.last_in_maps = in_maps

    y = np.empty((B,), np.float32)
    for ci in range(NCORES):
        y[ci * BC:(ci + 1) * BC] = np.asarray(
            res.results[ci]["yout"], np.float32)[0]
    return y
